# revision 16
# baseline (speedup 1.0000x reference)
"""AutomatonPELayer kernel for 8 Trainium2 NeuronCores.

Math: pe[j] = T^j @ x0 (j = 0..L-1), out = pe @ W.T + b, with T orthogonal
[128,128], L = 131072, embed dim 512, fp32.

Strategy (sequence-sharded):
- The output chunk of rows [128k, 128k+128) is B_k.T @ W.T where
  B_k = T^(128k) @ X and X = [x0, T x0, ..., T^127 x0]. Using
  B_{16g+j} = A_j' M_g' with A_j' = T^(128 j) X (anchor, fast index)
  and M_g' = T^(2048 g):   out_block(b=16g+j) = A_j'.T @ (M_g'.T W.T).
  j is the FAST block index so the kernel reaches full speed after
  loading just wgs[0] + the 16 anchors (~640 KB); the other 7 wgs
  slices stream in behind.
- Host (float64): per-core anchors A_j' (16 per core, advancing by
  T^128; core m offset by T^(16384 m)) and the 8 stride-folded weight
  matrices Wg = M_g'.T @ W.T. The device does ONLY 512-wide embed
  matmuls (fp16 operands, fp32 PSUM), a PSUM->SBUF convert, and the
  output DMA.
- Output is stored as int8 with a per-embed-column scale folded into
  Wg on the host (psum value = out/s_e), host decodes q * s_e. The
  per-column std is known analytically (T orthogonal => ||pe_row|| =
  ||x0|| is constant, so std(out[:,e]) ~= ||W_e||*||x0||/sqrt(128));
  scale covers C_SIGMA sigmas. This halves HBM write bytes vs fp16
  (8.39 MB/core) and quarters them vs fp32; rel err ~1.1e-2 against
  the 2e-2 gate. Set OUT_KIND="f16" for the conservative fallback
  (~2x bytes, rel err ~3e-4).
- b is folded in on the host during decode.
"""

import sys

if "/opt/trn_rl_repo" not in sys.path:
    sys.path.insert(0, "/opt/trn_rl_repo")

import numpy as np

L = 131072
S = 128  # num states (= partition dim = contraction dim)
E = 512  # embed dim
NCORES = 8
CHUNK = L // NCORES  # 16384 rows per core
BLOCKS = CHUNK // S  # 128 blocks of 128 rows per core
G = 8  # blocks per anchor group
GROUPS = BLOCKS // G  # 16 anchors per core
PAIRS = BLOCKS // 2  # matmul pairs sharing one PSUM tile
OCT = 8  # blocks per output store
C_SIGMA = 5.0  # int8 scale covers this many (predicted) sigmas

OUT_KIND = "i8"  # "i8" or "f16"

_prog_cache = {}


def _split_multi_waits(nc, mybir):
    """This walrus build accepts only ONE sync-wait per instruction
    (setupSyncWait: 'Too many sync wait commands'). Tile attaches the
    full wait list to the consuming instruction; hoist all but the
    last wait onto single-wait NoOps placed immediately before it on
    the same engine, preserving per-engine program order."""
    uid = 0
    for fn in nc.m.functions:
        for bb in fn.blocks:
            new = []
            changed = False
            for inst in bb.instructions:
                si = inst.sync_info
                waits = list(si.on_wait) if si is not None else []
                if len(waits) > 1:
                    changed = True
                    for w in waits[:-1]:
                        nop = mybir.InstNoOp(
                            name=f"splitw_{uid}",
                            engine=inst.engine,
                            sync_info=mybir.SyncInfo(on_wait=[w], on_update=[]),
                            bass_nofuse=True,
                        )
                        uid += 1
                        new.append(nop)
                    si.on_wait = [waits[-1]]
                new.append(inst)
            if changed:
                bb.instructions = new


def _copy_engine_seq(weights):
    """Largest-remainder rotation of the PSUM-drain paths.
    'v'/'s' = direct PSUM->int8 cast on DVE/ACT (1x, ~1.2us per pair).
    Only DVE and ACT can read PSUM (GPSIMD cannot; PE has no PSUM read
    port; int64 bitcast staging is illegal ISA on CoreV3), so the
    drain is bound to these two engines at 1 elem/lane/cycle."""
    tot = sum(weights.values())
    acc = {k: 0.0 for k in weights}
    seq = []
    for _ in range(PAIRS):
        for k in weights:
            acc[k] += weights[k] / tot
        pick = max(acc, key=lambda k: acc[k])
        acc[pick] -= 1.0
        seq.append(pick)
    return seq


def _build_program():
    key = ("nc", OUT_KIND)
    if key in _prog_cache:
        return _prog_cache[key]

    import concourse.bass as bass
    import concourse.tile as tile
    from concourse import mybir

    f32 = mybir.dt.float32
    f16 = mybir.dt.float16
    odt = mybir.dt.int8 if OUT_KIND == "i8" else f16
    nc = bass.Bass("TRN2", target_bir_lowering=False, debug=False, num_devices=NCORES)

    # anchors differ per core; wgs replicated (pre-scaled per column for i8).
    anchors = nc.dram_tensor("anchors", [GROUPS, S, S], f16, kind="ExternalInput").ap()
    wgs = nc.dram_tensor("wgs", [G, S, E], f16, kind="ExternalInput").ap()
    out = nc.dram_tensor("out", [CHUNK, E], odt, kind="ExternalOutput").ap()

    anchors_v = anchors.rearrange("j s i -> s j i")
    wgs_v = wgs.rearrange("g s e -> s g e")
    # Octo view: store t covers out rows [1024 t, 1024 t + 1024);
    # DRAM [t, p, b, e] matches an SBUF octo tile [p, b, e].
    out_v = out.rearrange("(t b p) e -> t p b e", b=OCT, p=S)
    out_pv = out.rearrange("(q b p) e -> q p b e", b=2, p=S)  # pair view

    # direct PSUM->int8 casts: DVE ~1.215us, ACT ~1.113us per pair
    eng_seq = _copy_engine_seq({"v": 1.0 / 1.215, "s": 1.0 / 1.113})

    with tile.TileContext(nc) as tc:
        with (
            tc.tile_pool(name="singles", bufs=1) as singles,
            tc.tile_pool(name="opool", bufs=3) as opool,
            tc.tile_pool(name="psum", bufs=4, space="PSUM") as psum,
        ):
            anch_t = singles.tile([S, GROUPS, S], f16)
            wgs_t = singles.tile([S, G, E], f16)
            dummy = singles.tile([S, 1], f32)
            # First-dependency loads issued in PARALLEL from both
            # HWDGE rings (the ~0.65us dma_start issue cost is the
            # ramp bottleneck): sync takes wgs[0], scalar takes the
            # first anchor, then one dummy ACT op triggers the
            # one-time ACT_TABLE_LOAD (~1.3us) off the critical path.
            # Remaining bulk loads ride the sync ring ahead of the
            # output stores (FIFO order matches readiness).
            nc.sync.dma_start(out=wgs_t[:, 0:1, :], in_=wgs_v[:, 0:1, :])
            nc.scalar.dma_start(out=anch_t[:, 0:2, :], in_=anchors_v[:, 0:2, :])
            nc.scalar.copy(out=dummy, in_=dummy)
            nc.sync.dma_start(out=anch_t[:, 2:GROUPS, :], in_=anchors_v[:, 2:GROUPS, :])
            nc.sync.dma_start(out=wgs_t[:, 1:8, :], in_=wgs_v[:, 1:8, :])

            NT = BLOCKS // OCT
            for t in range(NT):
                o_t = opool.tile([S, OCT, E], odt)
                for c in range(OCT // 2):  # pairs within the store
                    q = t * (OCT // 2) + c  # global pair index
                    pt = psum.tile([S, 2, E], f32)
                    for h in range(2):
                        k = 2 * q + h
                        g, j = divmod(k, GROUPS)
                        nc.tensor.matmul(
                            pt[:, h, :],
                            anch_t[:, j, :],
                            wgs_t[:, g, :],
                            start=True,
                            stop=True,
                        )
                    o_slice = o_t[:, 2 * c : 2 * c + 2, :]
                    if eng_seq[q] == "v":
                        nc.vector.tensor_copy(o_slice, pt)
                    else:
                        nc.scalar.copy(out=o_slice, in_=pt)
                # Final octo: pair-granular stores to shorten the tail.
                if t == NT - 1:
                    for c in range(OCT // 2):
                        q = t * (OCT // 2) + c
                        nc.sync.dma_start(
                            out=out_pv[q], in_=o_t[:, 2 * c : 2 * c + 2, :]
                        )
                else:
                    nc.sync.dma_start(out=out_v[t], in_=o_t)

    _split_multi_waits(nc, mybir)
    _prog_cache[key] = nc
    return nc


def _host_precompute(pos_initial, pos_transition, W):
    """float64 host prep: per-core anchor blocks + stride-folded weights
    (+ per-column int8 scales folded into the weights)."""
    T = np.asarray(pos_transition, np.float64)
    x0 = np.asarray(pos_initial, np.float64).reshape(S)
    W64 = np.asarray(W, np.float64)

    # X[:, i] = T^i x0 for i = 0..127 (exact sequential, f64)
    X = np.empty((S, S), np.float64)
    v = x0.copy()
    X[:, 0] = v
    for i in range(1, S):
        v = T @ v
        X[:, i] = v

    # T^128, T^2048, T^16384 by repeated squaring
    T128 = T.copy()
    for _ in range(7):
        T128 = T128 @ T128
    T2048 = T128.copy()
    for _ in range(4):
        T2048 = T2048 @ T2048
    T16384 = T2048 @ T2048
    T16384 = T16384 @ T16384
    T16384 = T16384 @ T16384

    # M_g' = T^(2048 g) for g = 0..G-1;  Wg = M_g'.T @ W.T  -> [G, S, E]
    Tp = [np.eye(S)]
    for g in range(1, G):
        Tp.append(Tp[-1] @ T2048)
    wgs = np.stack([np.ascontiguousarray(Tp[g].T @ W64.T) for g in range(G)])

    if OUT_KIND == "i8":
        # per-column scale: psum = out/s_e; int8 covers C_SIGMA sigmas of
        # the analytically-known column std (T orthogonal => constant
        # ||pe_row|| = ||x0||).
        sigma = np.linalg.norm(W64, axis=1) * np.linalg.norm(x0) / np.sqrt(S)
        scales = C_SIGMA * sigma / 127.0  # [E]
        wgs = wgs / scales[None, None, :]
    else:
        scales = None
    wgs = wgs.astype(np.float16)

    # Per-core, per-group anchors: A(m, j) = T^(16384 m + 128 j) @ X
    anchors = []
    B = X
    for _ in range(NCORES):
        steps = []
        A = B
        for _ in range(GROUPS):
            steps.append(A)
            A = T128 @ A
        anchors.append(np.asarray(steps, np.float64).astype(np.float16))
        B = T16384 @ B
    return anchors, wgs, scales


def _assemble(per_core_outs, scales, b):
    """Decode device outputs (int8 q -> q * s_e, or fp16 -> fp32) into
    the full fp32 [L, E] array."""
    full = np.concatenate(per_core_outs, axis=0)
    if OUT_KIND == "i8":
        full = full.astype(np.float32) * np.asarray(scales, np.float32)[None, :]
    else:
        full = full.astype(np.float32)
    b = np.asarray(b, np.float32)
    if np.any(b != 0):
        full = full + b[None, :]
    return np.ascontiguousarray(full)


def kernel(sentence_len, pos_initial, pos_transition, W, b):
    from concourse.bass_utils import run_bass_kernel_spmd

    assert int(sentence_len) == L, f"kernel hardcodes L={L}, got {sentence_len}"

    anchors, wgs, scales = _host_precompute(pos_initial, pos_transition, W)

    nc = _build_program()
    in_maps = [{"anchors": anchors[m], "wgs": wgs} for m in range(NCORES)]
    res = run_bass_kernel_spmd(nc, in_maps, core_ids=list(range(NCORES)))
    return _assemble([res.results[m]["out"] for m in range(NCORES)], scales, b)
